# revision 14
# baseline (speedup 1.0000x reference)
"""Trainium2 Bass kernel for nn_AttentionBlock (B=4, C=1024, T=2048, H=16, GROUPS=32).

Sharding: 8 cores = 4 batches x 2 T-halves. Each core computes, for its batch b
and query-half tq:
  - GroupNorm(x[b]) (full T, duplicated between the two half cores of a batch)
  - q = Wq@xn (local half cols), k/v = Wk/Wv@xn (full T)
  - masked softmax attention for all 16 heads, queries restricted to the half
  - proj + residual -> final out[b][:, half]  (no inter-core communication)
Host rotates T columns per core so the local query half is always columns
[0:1024) of the rotated frame; mask is rotated consistently on both axes.

Numerics: GroupNorm stats in fp32; xn, all weights, q/k/v, probabilities in
bf16 (matmuls accumulate fp32 in PSUM). Softmax denominators are produced by
the PV matmul itself (v^T tiles carry a 64-wide block of ones so the
denominator lands replicated on psum partitions 64:128); 1/d via Ln+Exp on
the scalar engine.
"""

import numpy as np
import ml_dtypes

import concourse.bass as bass
import concourse.bacc as bacc_mod
import concourse.tile as tile
import concourse.mybir as mybir
from concourse.bass_utils import run_bass_kernel_spmd

F32 = mybir.dt.float32
F32R = mybir.dt.float32r
BF16 = mybir.dt.bfloat16
FP8 = mybir.dt.float8e4
AF = mybir.ActivationFunctionType
OP = mybir.AluOpType
DR = mybir.MatmulPerfMode.DoubleRow
WSCALE = 8.0          # host pre-scales weights into fp8 normal range
ONES_V = 1.0 / 16.0   # ones block value; makes a_all land in fp8 normal range

B, C, T, H = 4, 1024, 2048, 16
GROUPS = 32
EPS = 1e-5
CH = C // H              # 64
SCALE = float(CH) ** -0.25
TQ = T // 2              # 1024 queries per core
NPAIR = H // 2           # 8 head pairs
NCO = C // 128           # 8 channel blocks
NTSB = T // 128          # 16 key/value blocks
INV_N = 1.0 / (32 * T)   # group size = 32 channels x T

# smalls tile column layout (all [128, 8] blocks)
S_S1, S_S2 = 0, 8
S_MU, S_E2, S_VAR, S_SQ, S_RS, S_A, S_B = 16, 24, 32, 40, 48, 56, 64
S_GNW, S_GNB, S_BQ, S_BK, S_BP = 72, 80, 88, 96, 104
S_EPS = 112
S_COLS = 113


def build_nc():
    nc = bacc_mod.Bacc(None, target_bir_lowering=False)
    f = {}
    f["x_t"] = nc.dram_tensor("x_t", [128, NCO, T], F32, kind="ExternalInput")
    f["mask_t"] = nc.dram_tensor("mask_t", [128, NTSB, TQ], BF16, kind="ExternalInput")
    f["wq_t"] = nc.dram_tensor("wq_t", [NPAIR, 128, NCO, 128], FP8, kind="ExternalInput")
    f["wk_t"] = nc.dram_tensor("wk_t", [NPAIR, 128, NCO, 128], FP8, kind="ExternalInput")
    f["wv_t"] = nc.dram_tensor("wv_t", [128, NCO, C], FP8, kind="ExternalInput")
    f["wp_t"] = nc.dram_tensor("wp_t", [NCO, 128, NCO, 128], FP8, kind="ExternalInput")
    f["gnw_t"] = nc.dram_tensor("gnw_t", [128, NCO], F32, kind="ExternalInput")
    f["gnb_t"] = nc.dram_tensor("gnb_t", [128, NCO], F32, kind="ExternalInput")
    f["bqs_t"] = nc.dram_tensor("bqs_t", [128, NPAIR], F32, kind="ExternalInput")
    f["bks_t"] = nc.dram_tensor("bks_t", [128, NPAIR], F32, kind="ExternalInput")
    f["bvb_t"] = nc.dram_tensor("bvb_t", [128, C], BF16, kind="ExternalInput")
    f["bp_t"] = nc.dram_tensor("bp_t", [128, NCO], F32, kind="ExternalInput")
    f["ind2_t"] = nc.dram_tensor("ind2_t", [128, 128], F32, kind="ExternalInput")
    out_t = nc.dram_tensor("out_t", [128, NCO, TQ], F32, kind="ExternalOutput")

    with tile.TileContext(nc) as tc:
        build_body(nc, tc, f, out_t)
    nc.compile()
    return nc


def build_body(nc, tc, f, out_t):
    import contextlib
    ctx = contextlib.ExitStack()
    with ctx:
        singles = ctx.enter_context(tc.tile_pool(name="singles", bufs=1))
        bigp = ctx.enter_context(tc.tile_pool(name="bigp", bufs=1))
        wqk = ctx.enter_context(tc.tile_pool(name="wqk", bufs=3))
        wvp = ctx.enter_context(tc.tile_pool(name="wvp", bufs=2))
        qpool = ctx.enter_context(tc.tile_pool(name="qpool", bufs=2))
        kpool = ctx.enter_context(tc.tile_pool(name="kpool", bufs=2))
        ppool = ctx.enter_context(tc.tile_pool(name="ppool", bufs=3))
        rpool = ctx.enter_context(tc.tile_pool(name="rpool", bufs=2))
        opool = ctx.enter_context(tc.tile_pool(name="opool", bufs=2))
        psum = ctx.enter_context(tc.tile_pool(name="psum", bufs=4, space="PSUM"))

        # ---- persistent tiles ----
        xnb = singles.tile([128, NCO, T], BF16)      # GroupNorm output (bf16)
        xn8 = singles.tile([128, NCO, T], FP8)       # GroupNorm output (fp8, matmul input)
        mask_s = singles.tile([128, NTSB, TQ], BF16)
        a_all = singles.tile([128, NPAIR, TQ], FP8)
        sm = singles.tile([128, S_COLS], F32)
        gst2 = singles.tile([128, 16], F32R)
        bvb = singles.tile([128, C], BF16)
        ind2_r = singles.tile([128, 128], F32)
        ind2 = singles.tile([128, 128], F32R)

        # raw x shares its SBUF slot with vt (v^T tiles), which is allocated
        # only after x is dead
        xs = bigp.tile([128, NCO, T], F32, tag="big", name="xs")

        for co in range(NCO):
            nc.sync.dma_start(out=xs[:, co, :], in_=f["x_t"][:, co, :])
        nc.sync.dma_start(out=mask_s, in_=f["mask_t"][:])
        nc.sync.dma_start(out=bvb, in_=f["bvb_t"][:])
        nc.sync.dma_start(out=ind2_r, in_=f["ind2_t"][:])
        nc.sync.dma_start(out=sm[:, S_GNW:S_GNW + 8], in_=f["gnw_t"][:])
        nc.sync.dma_start(out=sm[:, S_GNB:S_GNB + 8], in_=f["gnb_t"][:])
        nc.sync.dma_start(out=sm[:, S_BQ:S_BQ + 8], in_=f["bqs_t"][:])
        nc.sync.dma_start(out=sm[:, S_BK:S_BK + 8], in_=f["bks_t"][:])
        nc.sync.dma_start(out=sm[:, S_BP:S_BP + 8], in_=f["bp_t"][:])

        nc.vector.memset(sm[:, S_EPS:S_EPS + 1], EPS)

        # ---- GroupNorm stats ----
        # per-channel sums on DVE; per-channel sums of squares on Scalar
        # (Square + accumulator) so the two run concurrently
        trash = kpool.tile([128, T], BF16, tag="kpair", name="trash")
        for co in range(NCO):
            nc.vector.tensor_reduce(
                out=sm[:, co:co + 1], in_=xs[:, co, :], axis=mybir.AxisListType.X,
                op=OP.add,
            )
            nc.scalar.activation(
                out=trash, in_=xs[:, co, :], func=AF.Square,
                accum_out=sm[:, 8 + co:9 + co],
            )
        # group sums via indicator matmul: gps[p, j] = sum_m ind2[m, p] gst2[m, j]
        nc.vector.tensor_copy(out=gst2, in_=sm[:, 0:16])
        nc.vector.tensor_copy(out=ind2, in_=ind2_r)
        gps_full = psum.tile([128, 512], F32, tag="acc", bufs=2, name="gps")
        gps = gps_full[:, :16]
        nc.tensor.matmul(gps, lhsT=ind2, rhs=gst2,
                         start=True, stop=True)
        nc.vector.tensor_scalar_mul(sm[:, S_MU:S_MU + 8], gps[:, 0:8], INV_N)
        nc.vector.tensor_scalar_mul(sm[:, S_E2:S_E2 + 8], gps[:, 8:16], INV_N)
        nc.vector.tensor_tensor(sm[:, S_VAR:S_VAR + 8], sm[:, S_MU:S_MU + 8],
                                sm[:, S_MU:S_MU + 8], OP.mult)
        nc.vector.tensor_tensor(sm[:, S_VAR:S_VAR + 8], sm[:, S_E2:S_E2 + 8],
                                sm[:, S_VAR:S_VAR + 8], OP.subtract)
        nc.scalar.activation(out=sm[:, S_SQ:S_SQ + 8], in_=sm[:, S_VAR:S_VAR + 8],
                             func=AF.Sqrt, bias=sm[:, S_EPS:S_EPS + 1])
        nc.vector.reciprocal(out=sm[:, S_RS:S_RS + 8], in_=sm[:, S_SQ:S_SQ + 8])
        # A = rstd * gn_w ; Bc = gn_b - mu * A
        nc.vector.tensor_tensor(sm[:, S_A:S_A + 8], sm[:, S_RS:S_RS + 8],
                                sm[:, S_GNW:S_GNW + 8], OP.mult)
        nc.vector.tensor_tensor(sm[:, S_B:S_B + 8], sm[:, S_MU:S_MU + 8],
                                sm[:, S_A:S_A + 8], OP.mult)
        nc.vector.tensor_tensor(sm[:, S_B:S_B + 8], sm[:, S_GNB:S_GNB + 8],
                                sm[:, S_B:S_B + 8], OP.subtract)
        for co in range(NCO):
            nc.vector.tensor_scalar(
                out=xn8[:, co, :], in0=xs[:, co, :],
                scalar1=sm[:, S_A + co:S_A + co + 1],
                scalar2=sm[:, S_B + co:S_B + co + 1],
                op0=OP.mult, op1=OP.add,
            )
        for co in range(NCO):
            nc.vector.tensor_scalar(
                out=xnb[:, co, :], in0=xs[:, co, :],
                scalar1=sm[:, S_A + co:S_A + co + 1],
                scalar2=sm[:, S_B + co:S_B + co + 1],
                op0=OP.mult, op1=OP.add,
            )

        # ---- v^T for all heads (transposed-orientation matmul: lhsT = xn) ----
        # vt[:, tsb, h, 0:64] = v^T block; vt[:, tsb, h, 64:128] = ones so the
        # PV matmul also produces the softmax denominator on partitions 64:128
        vt = bigp.tile([128, NTSB, H, 128], BF16, tag="big", name="vt")
        nc.gpsimd.memset(vt[:, :, :, CH:128], ONES_V)

        def v_phase():
            for tbg in range(8):      # 8 groups of 2 t-blocks
                for rb in range(2):   # v-row halves (512 rows each = 8 heads)
                    vps = [psum.tile([128, 512], F32, tag="acc", bufs=2,
                                     name=f"vps{i}") for i in range(2)]
                    for kb2 in range(NCO // 2):
                        wv_sl = wvp.tile([128, 2, 512], FP8, tag="wv")
                        nc.gpsimd.dma_start(
                            out=wv_sl,
                            in_=f["wv_t"][:, 2 * kb2:2 * kb2 + 2, rb * 512:(rb + 1) * 512])
                        for i in range(2):
                            tb = tbg * 2 + i
                            nc.tensor.matmul(
                                vps[i],
                                lhsT=xn8[:, 2 * kb2:2 * kb2 + 2, tb * 128:(tb + 1) * 128],
                                rhs=wv_sl,
                                start=(kb2 == 0), stop=(kb2 == NCO // 2 - 1),
                                perf_mode=DR,
                            )
                    for i in range(2):
                        tb = tbg * 2 + i
                        nc.vector.tensor_tensor(
                            out=vt[:, tb, rb * 8:(rb + 1) * 8, 0:CH],
                            in0=vps[i].rearrange("p (h c) -> p h c", c=CH),
                            in1=bvb[:, rb * 512:(rb + 1) * 512].rearrange("p (h c) -> p h c", c=CH),
                            op=OP.add,
                        )

        # ---- per head-pair: q/k projections then attention ----
        # a_all normalization is deferred into the NEXT block's tsb loop so
        # the slow DVE reciprocal never blocks the boundary
        pending = []

        def emit_norm(item):
            accs_, lo_, hp_, tq_sl_ = item
            rd = rpool.tile([64, 512], F32, tag="rd", bufs=2, name="rd")
            nc.vector.reciprocal(out=rd, in_=accs_[64:128, :])
            nc.vector.tensor_tensor(
                out=a_all[lo_:lo_ + 64, hp_, tq_sl_], in0=accs_[0:64, :],
                in1=rd, op=OP.mult,
            )

        def make_qk(hp):
            """q/k projection for head-pair hp as chunks, so the work can be
            emitted inside the previous attention block's tail iterations
            (hiding it under the exp/mask pipeline)."""
            state = {}

            def c_dma():
                wq_sl = wqk.tile([128, NCO, 128], FP8, tag="wqkr", name="wq_sl")
                nc.gpsimd.dma_start(out=wq_sl, in_=f["wq_t"][hp])
                wk_sl = wqk.tile([128, NCO, 128], FP8, tag="wqkr", name="wk_sl")
                nc.gpsimd.dma_start(out=wk_sl, in_=f["wk_t"][hp])
                state["wq"], state["wk"] = wq_sl, wk_sl

            def c_q():
                q_pair = qpool.tile([128, TQ], BF16, tag="qpair", bufs=2,
                                    name="q_pair")
                state["q"] = q_pair
                qps2 = psum.tile([128, 1024], F32, tag="st", bufs=3, name="qps2")
                for tqb2 in range(2):
                    for kb2 in range(NCO // 2):
                        nc.tensor.matmul(
                            qps2[:, tqb2 * 512:(tqb2 + 1) * 512],
                            lhsT=state["wq"][:, 2 * kb2:2 * kb2 + 2, :],
                            rhs=xn8[:, 2 * kb2:2 * kb2 + 2,
                                    tqb2 * 512:(tqb2 + 1) * 512],
                            start=(kb2 == 0), stop=(kb2 == NCO // 2 - 1),
                            perf_mode=DR,
                        )
                nc.scalar.activation(
                    out=q_pair, in_=qps2, func=AF.Identity,
                    bias=sm[:, S_BQ + hp:S_BQ + hp + 1], scale=SCALE / WSCALE,
                )

            def c_k(th):
                if th == 0:
                    state["k"] = kpool.tile([128, T], BF16, tag="kpair",
                                            name="k_pair")
                k_pair = state["k"]
                kps2 = psum.tile([128, 1024], F32, tag="st", bufs=3, name="kps2")
                for tqb2 in range(2):
                    for kb2 in range(NCO // 2):
                        nc.tensor.matmul(
                            kps2[:, tqb2 * 512:(tqb2 + 1) * 512],
                            lhsT=state["wk"][:, 2 * kb2:2 * kb2 + 2, :],
                            rhs=xn8[:, 2 * kb2:2 * kb2 + 2,
                                    th * 1024 + tqb2 * 512:
                                    th * 1024 + (tqb2 + 1) * 512],
                            start=(kb2 == 0), stop=(kb2 == NCO // 2 - 1),
                            perf_mode=DR,
                        )
                nc.scalar.activation(
                    out=k_pair[:, th * 1024:(th + 1) * 1024], in_=kps2,
                    func=AF.Identity,
                    bias=sm[:, S_BK + hp:S_BK + hp + 1], scale=SCALE / WSCALE,
                )

            return state, [c_dma, c_q, lambda: c_k(0), lambda: c_k(1)]

        qk_state, chunks0 = make_qk(0)
        for c in chunks0:
            c()

        # prefetch the first two score blocks before the v-phase so the
        # exp/mask pipeline warms up while v is still being computed
        preseed = {}

        def scores_for(q_pair, k_pair, tqb, tsb):
            ts_sl = slice(tsb * 128, (tsb + 1) * 128)
            tq_sl_ = slice(tqb * 512, (tqb + 1) * 512)
            st2 = psum.tile([128, 1024], F32, tag="st", bufs=3, name="st2")
            for ih in range(2):
                nc.tensor.matmul(
                    st2[:, ih * 512:(ih + 1) * 512],
                    lhsT=k_pair[ih * 64:(ih + 1) * 64, ts_sl],
                    rhs=q_pair[ih * 64:(ih + 1) * 64, tq_sl_],
                    start=True, stop=True,
                )
            return st2

        preseed[(0, 0)] = [scores_for(qk_state["q"], qk_state["k"], 0, 0),
                           scores_for(qk_state["q"], qk_state["k"], 0, 1)]

        v_phase()

        for hp in range(NPAIR):
            q_pair, k_pair = qk_state["q"], qk_state["k"]
            next_state = None
            for tqb in range(2):
                last_block = (tqb == 1) and (hp + 1 < NPAIR)
                if last_block:
                    next_state, next_chunks = make_qk(hp + 1)
                else:
                    next_chunks = []
                tq_sl = slice(tqb * 512, (tqb + 1) * 512)

                def scores(tsb):
                    ts_sl = slice(tsb * 128, (tsb + 1) * 128)
                    st2 = psum.tile([128, 1024], F32, tag="st", bufs=3, name="st2")
                    for ih in range(2):
                        nc.tensor.matmul(
                            st2[:, ih * 512:(ih + 1) * 512],
                            lhsT=k_pair[ih * 64:(ih + 1) * 64, ts_sl],
                            rhs=q_pair[ih * 64:(ih + 1) * 64, tq_sl],
                            start=True, stop=True,
                        )
                    return st2

                acc_a = psum.tile([128, 512], F32, tag="acc", bufs=2, name="acc_a")
                acc_b = psum.tile([128, 512], F32, tag="acc", bufs=2, name="acc_b")
                sts = preseed.pop((hp, tqb), None) or [scores(0), scores(1)]
                for tsb in range(NTSB):
                    # prefetch two blocks ahead so Tensor never waits on
                    # the exp/mask chain
                    if tsb + 2 < NTSB:
                        sts.append(scores(tsb + 2))
                    st_cur = sts.pop(0)
                    pr2 = ppool.tile([128, 1024], BF16, tag="praw", bufs=3, name="pr2")
                    nc.scalar.activation(out=pr2, in_=st_cur, func=AF.Exp)
                    # mask the two heads on different engines (DVE + Pool),
                    # into separate tiles (single-writer keeps DVE 2x mode)
                    p_a = ppool.tile([128, 512], BF16, tag="pa", bufs=3, name="p_a")
                    nc.vector.tensor_tensor(
                        p_a, pr2[:, 0:512], mask_s[:, tsb, tq_sl], OP.mult)
                    p_b = ppool.tile([128, 512], BF16, tag="pb", bufs=3, name="p_b")
                    nc.gpsimd.tensor_tensor(
                        p_b, pr2[:, 512:1024], mask_s[:, tsb, tq_sl], OP.mult)
                    for ih, (acc, p) in ((0, (acc_a, p_a)), (1, (acc_b, p_b))):
                        nc.tensor.matmul(
                            acc, lhsT=vt[:, tsb, 2 * hp + ih, :],
                            rhs=p,
                            start=(tsb == 0), stop=(tsb == NTSB - 1),
                        )
                    if tsb in (3, 8) and pending:
                        emit_norm(pending.pop(0))
                    if tsb == NTSB - 1:
                        # free the accumulators with fast scalar copies
                        for ihead, acc in ((0, acc_a), (1, acc_b)):
                            accs = rpool.tile([128, 512], BF16, tag="accs",
                                              bufs=2, name="accs")
                            nc.scalar.activation(out=accs, in_=acc,
                                                 func=AF.Identity)
                            pending.append((accs, ihead * 64, hp, tq_sl))
                    # hide the next head-pair's q/k projections under the
                    # tail of this block (st-slot rotation frees up exactly
                    # here since the scores prefetch has ended)
                    if last_block:
                        if tsb == 8:
                            next_chunks[0]()
                        elif tsb == 13:
                            next_chunks[1]()
                        elif tsb == 14:
                            next_chunks[2]()
                        elif tsb == 15:
                            next_chunks[3]()
            if next_state is not None:
                qk_state = next_state

        while pending:
            emit_norm(pending.pop(0))

        # ---- proj + bias + residual ----
        for mb in range(NCO):
            wp_sl = wqk.tile([128, NCO, 128], FP8, tag="wqkr", name="wp_sl")
            nc.gpsimd.dma_start(out=wp_sl, in_=f["wp_t"][mb])
            for tqb in range(2):
                tq_sl = slice(tqb * 512, (tqb + 1) * 512)
                hps = psum.tile([128, 512], F32, tag="st", bufs=3, name="hps")
                for kb2 in range(NCO // 2):
                    nc.tensor.matmul(
                        hps, lhsT=wp_sl[:, 2 * kb2:2 * kb2 + 2, :],
                        rhs=a_all[:, 2 * kb2:2 * kb2 + 2, tq_sl],
                        start=(kb2 == 0), stop=(kb2 == NCO // 2 - 1),
                        perf_mode=DR,
                    )
                # hps = (8*Wp) @ (128*a) = 1024*h ; fold 1/1024 + bias here
                ot = opool.tile([128, 512], F32, tag="ot")
                nc.vector.tensor_scalar(
                    out=ot, in0=hps, scalar1=1.0 / 1024.0,
                    scalar2=sm[:, S_BP + mb:S_BP + mb + 1],
                    op0=OP.mult, op1=OP.add,
                )
                nc.vector.tensor_tensor(ot, ot, xnb[:, mb, tq_sl], OP.add)
                nc.sync.dma_start(out=out_t[:, mb, tq_sl], in_=ot)


_NC_CACHE = None


def _get_nc():
    global _NC_CACHE
    if _NC_CACHE is None:
        _NC_CACHE = build_nc()
    return _NC_CACHE


def _prep_inputs(x, mask, gn_weight, gn_bias, W_qkv, b_qkv, W_proj, b_proj):
    x = np.asarray(x, np.float32)
    mask = np.asarray(mask)
    gnw = np.asarray(gn_weight, np.float32)
    gnb = np.asarray(gn_bias, np.float32)
    W_qkv = np.asarray(W_qkv, np.float32)
    b_qkv = np.asarray(b_qkv, np.float32)
    W_proj = np.asarray(W_proj, np.float32)
    b_proj = np.asarray(b_proj, np.float32)

    Wh = W_qkv.reshape(H, 3, CH, C)
    bh = b_qkv.reshape(H, 3, CH)
    WqT = Wh[:, 0].reshape(C, C).T      # [c, qrow] head-major rows
    WkT = Wh[:, 1].reshape(C, C).T
    WvT = Wh[:, 2].reshape(C, C).T
    WpT = W_proj.T                       # [c_a, out_row]
    bq = bh[:, 0].reshape(C) * SCALE
    bk = bh[:, 1].reshape(C) * SCALE
    bv = bh[:, 2].reshape(C)

    def tile3(w):  # [C, N] -> [128, NCO, N] with c = co*128 + p
        return np.ascontiguousarray(w.reshape(NCO, 128, -1).transpose(1, 0, 2))

    wq3, wk3, wv3, wp3 = tile3(WqT), tile3(WkT), tile3(WvT), tile3(WpT)
    wq_t = (np.ascontiguousarray(
        np.stack([wq3[:, :, i * 128:(i + 1) * 128] for i in range(NPAIR)])
    ) * WSCALE).astype(ml_dtypes.float8_e4m3)
    wk_t = (np.ascontiguousarray(
        np.stack([wk3[:, :, i * 128:(i + 1) * 128] for i in range(NPAIR)])
    ) * WSCALE).astype(ml_dtypes.float8_e4m3)
    wp_t = (np.ascontiguousarray(
        np.stack([wp3[:, :, i * 128:(i + 1) * 128] for i in range(NCO)])
    ) * WSCALE).astype(ml_dtypes.float8_e4m3)

    col8 = lambda v: np.ascontiguousarray(v.reshape(8, 128).T)
    ind2 = np.zeros((128, 128), np.float32)
    for j in range(4):
        ind2[32 * j:32 * (j + 1), 32 * j:32 * (j + 1)] = 1.0
    common = {
        "wq_t": wq_t, "wk_t": wk_t,
        "wv_t": (np.ascontiguousarray(wv3) * WSCALE).astype(ml_dtypes.float8_e4m3),
        "wp_t": wp_t,
        "gnw_t": col8(gnw), "gnb_t": col8(gnb),
        "bqs_t": col8(bq), "bks_t": col8(bk),
        "bvb_t": np.ascontiguousarray(np.tile(bv[None, :] * WSCALE, (128, 1))).astype(ml_dtypes.bfloat16),
        "bp_t": col8(b_proj),
        "ind2_t": ind2,
    }

    maskT = mask.T.astype(np.float32)   # [ts, tq]
    in_maps = []
    for core in range(8):
        b, half = core // 2, core % 2
        perm = (np.arange(T) + half * TQ) % T
        x_c = x[b][:, perm]
        m_c = maskT[perm][:, perm[:TQ]]
        im = dict(common)
        im["x_t"] = np.ascontiguousarray(x_c.reshape(NCO, 128, T).transpose(1, 0, 2))
        im["mask_t"] = np.ascontiguousarray(
            m_c.reshape(NTSB, 128, TQ).transpose(1, 0, 2)).astype(ml_dtypes.bfloat16)
        in_maps.append(im)
    return in_maps


def _assemble(results):
    out = np.zeros((B, C, T), np.float32)
    for core, res in enumerate(results):
        b, half = core // 2, core % 2
        o = np.asarray(res["out_t"])                      # [128, NCO, TQ]
        out[b][:, half * TQ:(half + 1) * TQ] = o.transpose(1, 0, 2).reshape(C, TQ)
    return out


def run(inputs, trace=False, **kw):
    nc = _get_nc()
    in_maps = _prep_inputs(**inputs)
    br = run_bass_kernel_spmd(nc, in_maps, core_ids=list(range(8)), trace=trace, **kw)
    return _assemble(br.results), br


def kernel(**inputs):
    out, _ = run(inputs, trace=False)
    return out


# revision 17
# speedup vs baseline: 1.2119x; 1.2119x over previous
"""Trainium2 Bass kernel for nn_AttentionBlock (B=4, C=1024, T=2048, H=16, GROUPS=32).

Sharding: 8 cores = 4 batches x 2 T-halves. Each core computes, for its batch b
and query-half tq:
  - GroupNorm(x[b]) (full T, duplicated between the two half cores of a batch)
  - q = Wq@xn (local half cols), k/v = Wk/Wv@xn (full T)
  - masked softmax attention for all 16 heads, queries restricted to the half
  - proj + residual -> final out[b][:, half]  (no inter-core communication)
Host rotates T columns per core so the local query half is always columns
[0:1024) of the rotated frame; mask is rotated consistently on both axes.

Numerics: GroupNorm stats in fp32; xn, all weights, q/k/v, probabilities in
bf16 (matmuls accumulate fp32 in PSUM). Softmax denominators are produced by
the PV matmul itself (v^T tiles carry a 64-wide block of ones so the
denominator lands replicated on psum partitions 64:128); 1/d via Ln+Exp on
the scalar engine.
"""

import numpy as np
import ml_dtypes

import concourse.bass as bass
import concourse.bacc as bacc_mod
import concourse.tile as tile
import concourse.mybir as mybir
from concourse.bass_utils import run_bass_kernel_spmd

F32 = mybir.dt.float32
F32R = mybir.dt.float32r
BF16 = mybir.dt.bfloat16
FP8 = mybir.dt.float8e4
AF = mybir.ActivationFunctionType
OP = mybir.AluOpType
DR = mybir.MatmulPerfMode.DoubleRow
WSCALE = 8.0          # host pre-scales weights into fp8 normal range
ONES_V = 1.0 / 16.0   # ones block value; makes a_all land in fp8 normal range

B, C, T, H = 4, 1024, 2048, 16
GROUPS = 32
EPS = 1e-5
CH = C // H              # 64
SCALE = float(CH) ** -0.25
TQ = T // 2              # 1024 queries per core
NPAIR = H // 2           # 8 head pairs
NCO = C // 128           # 8 channel blocks
NTSB = T // 128          # 16 key/value blocks
INV_N = 1.0 / (32 * T)   # group size = 32 channels x T

# smalls tile column layout (all [128, 8] blocks)
S_S1, S_S2 = 0, 8
S_MU, S_E2, S_VAR, S_SQ, S_RS, S_A, S_B = 16, 24, 32, 40, 48, 56, 64
S_GNW, S_GNB, S_BQ, S_BK, S_BP = 72, 80, 88, 96, 104
S_EPS = 112
S_COLS = 113


def build_nc():
    nc = bacc_mod.Bacc(None, target_bir_lowering=False)
    f = {}
    f["x_t"] = nc.dram_tensor("x_t", [128, NCO, T], F32, kind="ExternalInput")
    f["mask_t"] = nc.dram_tensor("mask_t", [128, NTSB, TQ], BF16, kind="ExternalInput")
    f["wq_t"] = nc.dram_tensor("wq_t", [NPAIR, 128, NCO, 128], FP8, kind="ExternalInput")
    f["wk_t"] = nc.dram_tensor("wk_t", [NPAIR, 128, NCO, 128], FP8, kind="ExternalInput")
    f["wv_t"] = nc.dram_tensor("wv_t", [128, NCO, C], FP8, kind="ExternalInput")
    f["wp_t"] = nc.dram_tensor("wp_t", [NCO, 128, NCO, 128], FP8, kind="ExternalInput")
    f["gnw_t"] = nc.dram_tensor("gnw_t", [128, NCO], F32, kind="ExternalInput")
    f["gnb_t"] = nc.dram_tensor("gnb_t", [128, NCO], F32, kind="ExternalInput")
    f["bqs_t"] = nc.dram_tensor("bqs_t", [128, NPAIR], F32, kind="ExternalInput")
    f["bks_t"] = nc.dram_tensor("bks_t", [128, NPAIR], F32, kind="ExternalInput")
    f["bvb_t"] = nc.dram_tensor("bvb_t", [128, C], BF16, kind="ExternalInput")
    f["bp_t"] = nc.dram_tensor("bp_t", [128, NCO], F32, kind="ExternalInput")
    f["ind2_t"] = nc.dram_tensor("ind2_t", [128, 128], F32, kind="ExternalInput")
    out_t = nc.dram_tensor("out_t", [128, NCO, TQ], F32, kind="ExternalOutput")

    with tile.TileContext(nc) as tc:
        build_body(nc, tc, f, out_t)
    nc.compile()
    return nc


def build_body(nc, tc, f, out_t):
    import contextlib
    ctx = contextlib.ExitStack()
    with ctx:
        singles = ctx.enter_context(tc.tile_pool(name="singles", bufs=1))
        bigp = ctx.enter_context(tc.tile_pool(name="bigp", bufs=1))
        wqk = ctx.enter_context(tc.tile_pool(name="wqk", bufs=3))
        wvp = ctx.enter_context(tc.tile_pool(name="wvp", bufs=2))
        qpool = ctx.enter_context(tc.tile_pool(name="qpool", bufs=2))
        kpool = ctx.enter_context(tc.tile_pool(name="kpool", bufs=2))
        ppool = ctx.enter_context(tc.tile_pool(name="ppool", bufs=3))
        rpool = ctx.enter_context(tc.tile_pool(name="rpool", bufs=2))
        opool = ctx.enter_context(tc.tile_pool(name="opool", bufs=2))
        psum = ctx.enter_context(tc.tile_pool(name="psum", bufs=4, space="PSUM"))

        # ---- persistent tiles ----
        xnb = singles.tile([128, NCO, T], BF16)      # GroupNorm output (bf16)
        xn8 = singles.tile([128, NCO, T], FP8)       # GroupNorm output (fp8, matmul input)
        mask_s = singles.tile([128, NTSB, TQ], BF16)
        a_all = singles.tile([128, NPAIR, TQ], FP8)
        sm = singles.tile([128, S_COLS], F32)
        gst2 = singles.tile([128, 16], F32R)
        bvb = singles.tile([128, C], BF16)
        ind2_r = singles.tile([128, 128], F32)
        ind2 = singles.tile([128, 128], F32R)

        # raw x shares its SBUF slot with vt (v^T tiles), which is allocated
        # only after x is dead
        xs = bigp.tile([128, NCO, T], F32, tag="big", name="xs")

        for co in range(NCO):
            nc.sync.dma_start(out=xs[:, co, :], in_=f["x_t"][:, co, :])
        nc.sync.dma_start(out=mask_s, in_=f["mask_t"][:])
        nc.sync.dma_start(out=bvb, in_=f["bvb_t"][:])
        nc.sync.dma_start(out=ind2_r, in_=f["ind2_t"][:])
        nc.sync.dma_start(out=sm[:, S_GNW:S_GNW + 8], in_=f["gnw_t"][:])
        nc.sync.dma_start(out=sm[:, S_GNB:S_GNB + 8], in_=f["gnb_t"][:])
        nc.sync.dma_start(out=sm[:, S_BQ:S_BQ + 8], in_=f["bqs_t"][:])
        nc.sync.dma_start(out=sm[:, S_BK:S_BK + 8], in_=f["bks_t"][:])
        nc.sync.dma_start(out=sm[:, S_BP:S_BP + 8], in_=f["bp_t"][:])

        nc.vector.memset(sm[:, S_EPS:S_EPS + 1], EPS)

        # ---- GroupNorm stats ----
        # per-channel sums on DVE; per-channel sums of squares on Scalar
        # (Square + accumulator) so the two run concurrently
        trash = kpool.tile([128, T], BF16, tag="kpair", name="trash")
        for co in range(NCO):
            nc.vector.tensor_reduce(
                out=sm[:, co:co + 1], in_=xs[:, co, :], axis=mybir.AxisListType.X,
                op=OP.add,
            )
            nc.scalar.activation(
                out=trash, in_=xs[:, co, :], func=AF.Square,
                accum_out=sm[:, 8 + co:9 + co],
            )
        # group sums via indicator matmul: gps[p, j] = sum_m ind2[m, p] gst2[m, j]
        nc.vector.tensor_copy(out=gst2, in_=sm[:, 0:16])
        nc.vector.tensor_copy(out=ind2, in_=ind2_r)
        gps_full = psum.tile([128, 512], F32, tag="acc", bufs=2, name="gps")
        gps = gps_full[:, :16]
        nc.tensor.matmul(gps, lhsT=ind2, rhs=gst2,
                         start=True, stop=True)
        nc.vector.tensor_scalar_mul(sm[:, S_MU:S_MU + 8], gps[:, 0:8], INV_N)
        nc.vector.tensor_scalar_mul(sm[:, S_E2:S_E2 + 8], gps[:, 8:16], INV_N)
        nc.vector.tensor_tensor(sm[:, S_VAR:S_VAR + 8], sm[:, S_MU:S_MU + 8],
                                sm[:, S_MU:S_MU + 8], OP.mult)
        nc.vector.tensor_tensor(sm[:, S_VAR:S_VAR + 8], sm[:, S_E2:S_E2 + 8],
                                sm[:, S_VAR:S_VAR + 8], OP.subtract)
        nc.scalar.activation(out=sm[:, S_SQ:S_SQ + 8], in_=sm[:, S_VAR:S_VAR + 8],
                             func=AF.Sqrt, bias=sm[:, S_EPS:S_EPS + 1])
        nc.vector.reciprocal(out=sm[:, S_RS:S_RS + 8], in_=sm[:, S_SQ:S_SQ + 8])
        # A = rstd * gn_w ; Bc = gn_b - mu * A
        nc.vector.tensor_tensor(sm[:, S_A:S_A + 8], sm[:, S_RS:S_RS + 8],
                                sm[:, S_GNW:S_GNW + 8], OP.mult)
        nc.vector.tensor_tensor(sm[:, S_B:S_B + 8], sm[:, S_MU:S_MU + 8],
                                sm[:, S_A:S_A + 8], OP.mult)
        nc.vector.tensor_tensor(sm[:, S_B:S_B + 8], sm[:, S_GNB:S_GNB + 8],
                                sm[:, S_B:S_B + 8], OP.subtract)
        for co in range(NCO):
            nc.vector.tensor_scalar(
                out=xn8[:, co, :], in0=xs[:, co, :],
                scalar1=sm[:, S_A + co:S_A + co + 1],
                scalar2=sm[:, S_B + co:S_B + co + 1],
                op0=OP.mult, op1=OP.add,
            )
        for co in range(NCO):
            nc.gpsimd.tensor_scalar(
                out=xnb[:, co, :], in0=xs[:, co, :],
                scalar1=sm[:, S_A + co:S_A + co + 1],
                scalar2=sm[:, S_B + co:S_B + co + 1],
                op0=OP.mult, op1=OP.add,
            )

        # ---- v^T for all heads (transposed-orientation matmul: lhsT = xn) ----
        # vt[:, tsb, h, 0:64] = v^T block; vt[:, tsb, h, 64:128] = ones so the
        # PV matmul also produces the softmax denominator on partitions 64:128
        vt = bigp.tile([128, NTSB, H, 128], BF16, tag="big", name="vt")
        nc.gpsimd.memset(vt[:, :, :, CH:128], ONES_V)

        for tbg in range(4):          # 4 groups of 4 t-blocks
            for rb in range(2):       # v-row halves (512 rows each = 8 heads)
                vps = [psum.tile([128, 512], F32, tag=("acc" if i < 2 else "st"),
                                 bufs=(2 if i < 2 else 3), name=f"vps{i}")
                       for i in range(4)]
                for kb2 in range(NCO // 2):
                    wv_sl = wvp.tile([128, 2, 512], FP8, tag="wv")
                    nc.gpsimd.dma_start(
                        out=wv_sl,
                        in_=f["wv_t"][:, 2 * kb2:2 * kb2 + 2, rb * 512:(rb + 1) * 512])
                    for i in range(4):
                        tb = tbg * 4 + i
                        nc.tensor.matmul(
                            vps[i],
                            lhsT=xn8[:, 2 * kb2:2 * kb2 + 2, tb * 128:(tb + 1) * 128],
                            rhs=wv_sl,
                            start=(kb2 == 0), stop=(kb2 == NCO // 2 - 1),
                            perf_mode=DR,
                        )
                for i in range(4):
                    tb = tbg * 4 + i
                    nc.vector.tensor_tensor(
                        out=vt[:, tb, rb * 8:(rb + 1) * 8, 0:CH],
                        in0=vps[i].rearrange("p (h c) -> p h c", c=CH),
                        in1=bvb[:, rb * 512:(rb + 1) * 512].rearrange("p (h c) -> p h c", c=CH),
                        op=OP.add,
                    )

        # ---- per head-pair: q/k projections then attention ----
        # a_all normalization is deferred into the NEXT block's tsb loop so
        # the slow DVE reciprocal never blocks the boundary
        pending = []

        def emit_norm(item):
            # -1/d via bf16 bit-hack seed + one Newton step, in cheap DVE
            # ALU ops (the iterative InstReciprocal is 4x slower); the sign
            # is folded into W_proj on the host
            accs_, lo_, hp_, tq_sl_ = item
            I16 = mybir.dt.int16
            # scratch tiles are [128, 512] sliced at [64:128] so SB+SB
            # operands share the same base partition as accs_[64:128]
            t0f = rpool.tile([128, 512], I16, tag="rd", bufs=4, name="t0f")
            t0 = t0f[64:128, :]
            nc.vector.tensor_scalar(
                out=t0, in0=accs_[64:128, :].bitcast(I16),
                scalar1=0x7EF3, scalar2=None, op0=OP.subtract,
            )
            t1f = rpool.tile([128, 512], I16, tag="rd", bufs=4, name="t1f")
            t1 = t1f[64:128, :]
            nc.vector.tensor_scalar(
                out=t1, in0=t0, scalar1=-1, scalar2=None, op0=OP.bitwise_xor,
            )
            r0 = t1.bitcast(BF16)
            uf = rpool.tile([128, 512], BF16, tag="rd", bufs=4, name="uf")
            u = uf[64:128, :]
            nc.vector.tensor_tensor(out=u, in0=accs_[64:128, :], in1=r0,
                                    op=OP.mult)
            wf = rpool.tile([128, 512], BF16, tag="rd", bufs=4, name="wf")
            w = wf[64:128, :]
            nc.vector.tensor_scalar(
                out=w, in0=u, scalar1=2.0, scalar2=None, op0=OP.subtract)
            rnegf = rpool.tile([128, 512], BF16, tag="rd", bufs=4, name="rnegf")
            rneg = rnegf[0:64, :]
            nc.vector.tensor_tensor(out=rneg, in0=r0, in1=w, op=OP.mult)
            nc.gpsimd.tensor_tensor(
                out=a_all[lo_:lo_ + 64, hp_, tq_sl_], in0=accs_[0:64, :],
                in1=rneg, op=OP.mult,
            )

        def make_qk(hp):
            """q/k projection for head-pair hp as chunks, so the work can be
            emitted inside the previous attention block's tail iterations
            (hiding it under the exp/mask pipeline)."""
            state = {}

            def c_dma():
                wq_sl = wqk.tile([128, NCO, 128], FP8, tag="wqkr", name="wq_sl")
                nc.gpsimd.dma_start(out=wq_sl, in_=f["wq_t"][hp])
                wk_sl = wqk.tile([128, NCO, 128], FP8, tag="wqkr", name="wk_sl")
                nc.gpsimd.dma_start(out=wk_sl, in_=f["wk_t"][hp])
                state["wq"], state["wk"] = wq_sl, wk_sl

            def c_q():
                q_pair = qpool.tile([128, TQ], BF16, tag="qpair", bufs=2,
                                    name="q_pair")
                state["q"] = q_pair
                qps2 = psum.tile([128, 1024], F32, tag="st", bufs=3, name="qps2")
                for tqb2 in range(2):
                    for kb2 in range(NCO // 2):
                        nc.tensor.matmul(
                            qps2[:, tqb2 * 512:(tqb2 + 1) * 512],
                            lhsT=state["wq"][:, 2 * kb2:2 * kb2 + 2, :],
                            rhs=xn8[:, 2 * kb2:2 * kb2 + 2,
                                    tqb2 * 512:(tqb2 + 1) * 512],
                            start=(kb2 == 0), stop=(kb2 == NCO // 2 - 1),
                            perf_mode=DR,
                        )
                nc.scalar.activation(
                    out=q_pair, in_=qps2, func=AF.Identity,
                    bias=sm[:, S_BQ + hp:S_BQ + hp + 1], scale=SCALE / WSCALE,
                )

            def c_k(th):
                if th == 0:
                    state["k"] = kpool.tile([128, T], BF16, tag="kpair",
                                            name="k_pair")
                k_pair = state["k"]
                kps2 = psum.tile([128, 1024], F32, tag="st", bufs=3, name="kps2")
                for tqb2 in range(2):
                    for kb2 in range(NCO // 2):
                        nc.tensor.matmul(
                            kps2[:, tqb2 * 512:(tqb2 + 1) * 512],
                            lhsT=state["wk"][:, 2 * kb2:2 * kb2 + 2, :],
                            rhs=xn8[:, 2 * kb2:2 * kb2 + 2,
                                    th * 1024 + tqb2 * 512:
                                    th * 1024 + (tqb2 + 1) * 512],
                            start=(kb2 == 0), stop=(kb2 == NCO // 2 - 1),
                            perf_mode=DR,
                        )
                nc.scalar.activation(
                    out=k_pair[:, th * 1024:(th + 1) * 1024], in_=kps2,
                    func=AF.Identity,
                    bias=sm[:, S_BK + hp:S_BK + hp + 1], scale=SCALE / WSCALE,
                )

            return state, [c_dma, c_q, lambda: c_k(0), lambda: c_k(1)]

        qk_state, chunks0 = make_qk(0)
        for c in chunks0:
            c()

        for hp in range(NPAIR):
            q_pair, k_pair = qk_state["q"], qk_state["k"]
            next_state = None
            for tqb in range(2):
                last_block = (tqb == 1) and (hp + 1 < NPAIR)
                if last_block:
                    next_state, next_chunks = make_qk(hp + 1)
                else:
                    next_chunks = []
                tq_sl = slice(tqb * 512, (tqb + 1) * 512)

                def scores(tsb):
                    ts_sl = slice(tsb * 128, (tsb + 1) * 128)
                    st2 = psum.tile([128, 1024], F32, tag="st", bufs=3, name="st2")
                    for ih in range(2):
                        nc.tensor.matmul(
                            st2[:, ih * 512:(ih + 1) * 512],
                            lhsT=k_pair[ih * 64:(ih + 1) * 64, ts_sl],
                            rhs=q_pair[ih * 64:(ih + 1) * 64, tq_sl],
                            start=True, stop=True,
                        )
                    return st2

                acc_a = psum.tile([128, 512], F32, tag="acc", bufs=2, name="acc_a")
                acc_b = psum.tile([128, 512], F32, tag="acc", bufs=2, name="acc_b")
                sts = [scores(0), scores(1)]
                for tsb in range(NTSB):
                    # prefetch two blocks ahead so Tensor never waits on
                    # the exp/mask chain
                    if tsb + 2 < NTSB:
                        sts.append(scores(tsb + 2))
                    st_cur = sts.pop(0)
                    pr2 = ppool.tile([128, 1024], BF16, tag="praw", bufs=3, name="pr2")
                    nc.scalar.activation(out=pr2, in_=st_cur, func=AF.Exp)
                    # one DVE op masks both heads (mask broadcast via
                    # zero-stride middle axis)
                    p2 = ppool.tile([128, 1024], BF16, tag="p", bufs=3, name="p2")
                    pr3 = pr2.rearrange("p (h q) -> p h q", h=2)
                    m3 = mask_s[:, tsb, tq_sl].rearrange("p (o q) -> p o q", o=1)
                    m3b, _ = bass.broadcast_tensor_aps(m3, pr3)
                    nc.vector.tensor_tensor(
                        p2.rearrange("p (h q) -> p h q", h=2), pr3, m3b, OP.mult)
                    for ih, acc in ((0, acc_a), (1, acc_b)):
                        nc.tensor.matmul(
                            acc, lhsT=vt[:, tsb, 2 * hp + ih, :],
                            rhs=p2[:, ih * 512:(ih + 1) * 512],
                            start=(tsb == 0), stop=(tsb == NTSB - 1),
                        )
                    if tsb in (3, 8) and pending:
                        emit_norm(pending.pop(0))
                    if tsb == NTSB - 1:
                        # free the accumulators with fast scalar copies
                        for ihead, acc in ((0, acc_a), (1, acc_b)):
                            accs = rpool.tile([128, 512], BF16, tag="accs",
                                              bufs=2, name="accs")
                            nc.scalar.activation(out=accs, in_=acc,
                                                 func=AF.Identity)
                            pending.append((accs, ihead * 64, hp, tq_sl))
                    # hide the next head-pair's q/k projections under the
                    # tail of this block (st-slot rotation frees up exactly
                    # here since the scores prefetch has ended)
                    if last_block:
                        if tsb == 8:
                            next_chunks[0]()
                        elif tsb == 13:
                            next_chunks[1]()
                        elif tsb == 14:
                            next_chunks[2]()
                        elif tsb == 15:
                            next_chunks[3]()
            if next_state is not None:
                qk_state = next_state

        while pending:
            emit_norm(pending.pop(0))

        # ---- proj + bias + residual ----
        for mb in range(NCO):
            wp_sl = wqk.tile([128, NCO, 128], FP8, tag="wqkr", name="wp_sl")
            nc.gpsimd.dma_start(out=wp_sl, in_=f["wp_t"][mb])
            for tqb in range(2):
                tq_sl = slice(tqb * 512, (tqb + 1) * 512)
                hps = psum.tile([128, 512], F32, tag="st", bufs=3, name="hps")
                for kb2 in range(NCO // 2):
                    nc.tensor.matmul(
                        hps, lhsT=wp_sl[:, 2 * kb2:2 * kb2 + 2, :],
                        rhs=a_all[:, 2 * kb2:2 * kb2 + 2, tq_sl],
                        start=(kb2 == 0), stop=(kb2 == NCO // 2 - 1),
                        perf_mode=DR,
                    )
                # hps = (8*Wp) @ (128*a) = 1024*h ; fold 1/1024 + bias here
                ot = opool.tile([128, 512], F32, tag="ot")
                nc.vector.tensor_scalar(
                    out=ot, in0=hps, scalar1=1.0 / 1024.0,
                    scalar2=sm[:, S_BP + mb:S_BP + mb + 1],
                    op0=OP.mult, op1=OP.add,
                )
                nc.vector.tensor_tensor(ot, ot, xnb[:, mb, tq_sl], OP.add)
                nc.sync.dma_start(out=out_t[:, mb, tq_sl], in_=ot)


_NC_CACHE = None


def _get_nc():
    global _NC_CACHE
    if _NC_CACHE is None:
        _NC_CACHE = build_nc()
    return _NC_CACHE


def _prep_inputs(x, mask, gn_weight, gn_bias, W_qkv, b_qkv, W_proj, b_proj):
    x = np.asarray(x, np.float32)
    mask = np.asarray(mask)
    gnw = np.asarray(gn_weight, np.float32)
    gnb = np.asarray(gn_bias, np.float32)
    W_qkv = np.asarray(W_qkv, np.float32)
    b_qkv = np.asarray(b_qkv, np.float32)
    W_proj = np.asarray(W_proj, np.float32)
    b_proj = np.asarray(b_proj, np.float32)

    Wh = W_qkv.reshape(H, 3, CH, C)
    bh = b_qkv.reshape(H, 3, CH)
    WqT = Wh[:, 0].reshape(C, C).T      # [c, qrow] head-major rows
    WkT = Wh[:, 1].reshape(C, C).T
    WvT = Wh[:, 2].reshape(C, C).T
    WpT = W_proj.T                       # [c_a, out_row]
    bq = bh[:, 0].reshape(C) * SCALE
    bk = bh[:, 1].reshape(C) * SCALE
    bv = bh[:, 2].reshape(C)

    def tile3(w):  # [C, N] -> [128, NCO, N] with c = co*128 + p
        return np.ascontiguousarray(w.reshape(NCO, 128, -1).transpose(1, 0, 2))

    wq3, wk3, wv3, wp3 = tile3(WqT), tile3(WkT), tile3(WvT), tile3(WpT)
    wq_t = (np.ascontiguousarray(
        np.stack([wq3[:, :, i * 128:(i + 1) * 128] for i in range(NPAIR)])
    ) * WSCALE).astype(ml_dtypes.float8_e4m3)
    wk_t = (np.ascontiguousarray(
        np.stack([wk3[:, :, i * 128:(i + 1) * 128] for i in range(NPAIR)])
    ) * WSCALE).astype(ml_dtypes.float8_e4m3)
    # negative scale folds the Newton-reciprocal sign into the projection
    wp_t = (np.ascontiguousarray(
        np.stack([wp3[:, :, i * 128:(i + 1) * 128] for i in range(NCO)])
    ) * -WSCALE).astype(ml_dtypes.float8_e4m3)

    col8 = lambda v: np.ascontiguousarray(v.reshape(8, 128).T)
    ind2 = np.zeros((128, 128), np.float32)
    for j in range(4):
        ind2[32 * j:32 * (j + 1), 32 * j:32 * (j + 1)] = 1.0
    common = {
        "wq_t": wq_t, "wk_t": wk_t,
        "wv_t": (np.ascontiguousarray(wv3) * WSCALE).astype(ml_dtypes.float8_e4m3),
        "wp_t": wp_t,
        "gnw_t": col8(gnw), "gnb_t": col8(gnb),
        "bqs_t": col8(bq), "bks_t": col8(bk),
        "bvb_t": np.ascontiguousarray(np.tile(bv[None, :] * WSCALE, (128, 1))).astype(ml_dtypes.bfloat16),
        "bp_t": col8(b_proj),
        "ind2_t": ind2,
    }

    maskT = mask.T.astype(np.float32)   # [ts, tq]
    in_maps = []
    for core in range(8):
        b, half = core // 2, core % 2
        perm = (np.arange(T) + half * TQ) % T
        x_c = x[b][:, perm]
        m_c = maskT[perm][:, perm[:TQ]]
        im = dict(common)
        im["x_t"] = np.ascontiguousarray(x_c.reshape(NCO, 128, T).transpose(1, 0, 2))
        im["mask_t"] = np.ascontiguousarray(
            m_c.reshape(NTSB, 128, TQ).transpose(1, 0, 2)).astype(ml_dtypes.bfloat16)
        in_maps.append(im)
    return in_maps


def _assemble(results):
    out = np.zeros((B, C, T), np.float32)
    for core, res in enumerate(results):
        b, half = core // 2, core % 2
        o = np.asarray(res["out_t"])                      # [128, NCO, TQ]
        out[b][:, half * TQ:(half + 1) * TQ] = o.transpose(1, 0, 2).reshape(C, TQ)
    return out


def run(inputs, trace=False, **kw):
    nc = _get_nc()
    in_maps = _prep_inputs(**inputs)
    br = run_bass_kernel_spmd(nc, in_maps, core_ids=list(range(8)), trace=trace, **kw)
    return _assemble(br.results), br


def kernel(**inputs):
    out, _ = run(inputs, trace=False)
    return out
